# revision 16
# baseline (speedup 1.0000x reference)
"""Chamfer-KL loss kernel for Trainium2 (Bass/Tile), exact-min design.

Math (per batch element b, core b):
    T[x,y] = pm2 + c[y] + r[x],  pm2 = t_var + t_mua - 2 t_cross
      c[y] = sum_lb[y] + t_mub[y],  r[x] = -sum_la[x]
    p_kl = 0.5*(T - d)
    loss = 0.5*[ sum_y (min_x T - d) + sum_x mask[x]*(min_y T - d) ]

GEMM: 2 bf16 K=128 matmuls per 512-col PSUM half:
      L1 = (exp(la) + mu_a^2)^T   R1 = exp(-lb)^T
      L2 = (-2 mu_a)^T            R2 = (mu_b * exp(-lb))^T
plus, for K3 of the 32 x-tiles, a 3rd matmul (L3=[1;sum_la], R3=[c-CM;-1])
that completes T in PSUM ("3mm" tiles).  The mix balances PE against DVE:
3mm tiles pay PE but get a cheap epilogue; 2mm tiles skip the 3rd matmul
and add c via a DVE tensor add against the broadcast tile cbc.

Epilogue per [P,1024] PSUM tile (TREF centering keeps bf16 exact-ish):
    cp = Act Identity(pm2) + bias        (bias supplies r and/or CM-TREF)
    [2mm] tfull = cp + cbc               (DVE tt, 2x)
    rows: DVE tensor_scalar min w/ accum (4x mode, 327ns)
    cols: DVE tt min into running colmin (2x mode)
Exact mins both directions - no softmin, no exp epilogue.

Input staging: gpsimd casting DMA (f32 HBM -> bf16 SBUF), transposes via
DMA-transpose XBAR (bf16 SBUF->SBUF) - nothing on PE but matmuls.

Sharding: data-parallel over batch; core i handles batch element i fully.
"""

import os
import numpy as np

import concourse.bass as bass
import concourse.tile as tile
from concourse import mybir
from concourse.bass_utils import run_bass_kernel_spmd
from concourse.masks import make_identity

F32 = mybir.dt.float32
F32R = mybir.dt.float32r
BF16 = mybir.dt.bfloat16
AX = mybir.AxisListType
OP = mybir.AluOpType
AF = mybir.ActivationFunctionType

BS, NX, NY, D = 8, 4096, 4096, 128
P = 128      # SBUF partitions
YB = 512     # one PSUM bank of fp32
GT = 4       # y tiles per B-side feature group
BIG = 1e30
TREF = 512.0   # T centering so bf16 tiles are accurate near the mins
CM = 330.0     # c centering inside R3/cbc

K3 = 26        # x-tiles (of nt) carrying the 3rd (bias) matmul
PAIR = 2       # PSUM banks per epilogue tile


def _dbg(name, default):
    if bool(int(os.environ.get("KERN_DEBUG", "0"))):
        return os.environ.get(name, default)
    return default


def _body(tc, mu_a, la, mu_b, lb, mask, out_d, nx, ny, repeat=1):
    nc = tc.nc
    nt = nx // P     # x tiles
    nu = ny // P     # y tiles
    nyb = ny // YB   # y blocks of 512
    k3 = int(_dbg("KERN_K3", str(K3)))
    pair = int(_dbg("KERN_PAIR", str(PAIR)))
    sqa = bool(int(_dbg("KERN_SQA", "1")))   # square on Act (vs DVE)
    assert nyb % pair == 0
    nyp = nyb // pair

    LEAD2 = min(6, nt - k3)  # leading 2mm tiles: GEMM starts before R3 lands

    def is3(t):
        if t < LEAD2:
            return False
        # Bresenham spread of k3 three-matmul tiles over the rest
        n_rest = nt - LEAD2
        tt = t - LEAD2
        return ((tt + 1) * k3) // n_rest > (tt * k3) // n_rest

    with tc.tile_pool(name="const", bufs=1) as const:
        ident = const.tile([P, P], F32)
        make_identity(nc, ident)
        ident_e = const.tile([P, P], BF16)
        nc.vector.tensor_copy(ident_e, ident)
        ones_f = const.tile([P, 1], F32)
        nc.vector.memset(ones_f, 1.0)
        ones_sq = const.tile([P, P], BF16)
        nc.vector.memset(ones_sq, 1.0)
        negcm = const.tile([P, 1], F32)
        nc.vector.memset(negcm, -CM)
        bias3 = const.tile([P, 1], F32)
        nc.vector.memset(bias3, CM - TREF)

        nbuf = 2 if repeat > 1 else 1
        L1s = [const.tile([P, nx], BF16, name=f"L1_{i}") for i in range(nbuf)]
        L2s = [const.tile([P, nx], BF16, name=f"L2_{i}") for i in range(nbuf)]
        R1s = [const.tile([P, ny], BF16, name=f"R1_{i}") for i in range(nbuf)]
        R2s = [const.tile([P, ny], BF16, name=f"R2_{i}") for i in range(nbuf)]
        cbcs = [const.tile([P, ny], BF16, name=f"cbc_{i}") for i in range(nbuf)]
        colmins = [const.tile([P, ny], BF16, name="colmin_0")] * nbuf
        rowmins = [const.tile([P, nt], F32, name=f"rowmin_{i}") for i in range(nbuf)]
        sumlas = [const.tile([P, nt], F32, name=f"sumla_{i}") for i in range(nbuf)]
        nbiases = [const.tile([P, nt], F32, name=f"nbias_{i}") for i in range(nbuf)]
        slotss = [const.tile([P, nt * nyp], F32, name=f"slots_{i}") for i in range(nbuf)]
        masks = [const.tile([P, nt], F32, name=f"mask_{i}") for i in range(nbuf)]

        use3 = k3 > 0
        if use3:
            # K padded to 128: small-K matmuls pay a fixed penalty on HW
            # L3 row0 = 1, row1 = sum_la (DMA'd later), rows 2+ = 0.
            # R3 row0 = c - CM (copied later), row1 = -1, rows 2+ = 0.
            L3s = [const.tile([P, nx], BF16, name=f"L3_{i}") for i in range(nbuf)]
            R3s = [const.tile([P, ny], BF16, name=f"R3_{i}") for i in range(nbuf)]
            init_p = const.tile([2, YB], F32)
            nc.vector.memset(init_p, 1.0)
            init_n = const.tile([2, YB], F32)
            nc.vector.memset(init_n, -1.0)
            for L3 in L3s:
                nc.vector.memset(L3, 0.0)
                for z in range(0, nx, YB):
                    nc.scalar.copy(L3[0:2, z : z + YB], init_p)
            for R3 in R3s:
                nc.vector.memset(R3, 0.0)
                for z in range(0, ny, YB):
                    nc.scalar.copy(R3[0:2, z : z + YB], init_n)
        else:
            L3s, R3s = [None] * nbuf, [None] * nbuf

        def _phases(R1, R2, R3, cbc, L1, L2, L3, colmin, rowmin_all,
                    sumla_nat, nbias, slots_all, mask_sb):
            ct = min(16, nt)  # tiles per DMA chunk
            nc.vector.memset(colmin, BIG)
            with (
                tc.tile_pool(name="big", bufs=2) as big,
                tc.tile_pool(name="sc", bufs=4) as sc,
                tc.tile_pool(name="pso", bufs=1, space="PSUM") as pso,
                tc.tile_pool(name="psm", bufs=2, space="PSUM") as psm,
                tc.tile_pool(name="bfp", bufs=6) as bfp,
            ):
                def a_prep(c):
                    rows = slice(c * ct * P, (c + 1) * ct * P)
                    csl = slice(c * ct, (c + 1) * ct)
                    dla = big.tile([P, ct, D], BF16, tag="bigA", bufs=2, name=f"dla{c}")
                    nc.gpsimd.dma_start(
                        out=dla, in_=la[rows, :].rearrange("(p t) f -> p t f", p=P)
                    )
                    dmaa = big.tile([P, ct, D], BF16, tag="bigA", bufs=2, name=f"dmaa{c}")
                    nc.gpsimd.dma_start(
                        out=dmaa, in_=mu_a[rows, :].rearrange("(p t) f -> p t f", p=P)
                    )
                    nc.vector.tensor_reduce(
                        sumla_nat[:, csl], dla, axis=AX.X, op=OP.add
                    )
                    # 2mm Act bias: r - TREF + CM = -sum_la + (CM - TREF)
                    nc.vector.tensor_scalar(
                        out=nbias[:, csl], in0=sumla_nat[:, csl],
                        scalar1=-1.0, scalar2=CM - TREF,
                        op0=OP.mult, op1=OP.add,
                    )
                    e_nat = sc.tile([P, ct, D], BF16, tag="ha", bufs=3)
                    nc.scalar.activation(e_nat, dla, AF.Exp)
                    sq_nat = sc.tile([P, ct, D], BF16, tag="ha", bufs=3)
                    if sqa:
                        nc.scalar.activation(sq_nat, dmaa, AF.Square)
                    else:
                        nc.vector.tensor_mul(sq_nat, dmaa, dmaa)
                    l1_nat = sc.tile([P, ct, D], BF16, tag="ha", bufs=3)
                    nc.vector.tensor_add(l1_nat, e_nat, sq_nat)
                    l2_nat = sc.tile([P, ct, D], BF16, tag="ha", bufs=3)
                    nc.vector.tensor_scalar_mul(l2_nat, dmaa, -2.0)
                    xsc = slice(c * ct * P, (c + 1) * ct * P)
                    nc.sync.dma_start(
                        out=L1[:, xsc].rearrange("p (t f) -> p t f", t=ct),
                        in_=l1_nat.rearrange("p t f -> p (t f)"),
                        transpose=True,
                    )
                    nc.sync.dma_start(
                        out=L2[:, xsc].rearrange("p (t f) -> p t f", t=ct),
                        in_=l2_nat.rearrange("p t f -> p (t f)"),
                        transpose=True,
                    )
                    if use3:
                        # L3 row1 chunk = sum_la chunk, transposed to free dim
                        p_slc = pso.tile([ct, P], F32, tag="po", bufs=2)
                        nc.tensor.transpose(p_slc, sumla_nat[:, csl], ident)
                        sla_c = sc.tile([ct, P], BF16, tag="sc2")
                        nc.vector.tensor_copy(sla_c, p_slc)
                        nc.sync.dma_start(
                            out=L3[1:2, c * ct * P : (c + 1) * ct * P].rearrange(
                                "p (t f) -> p t f", t=ct
                            ),
                            in_=sla_c,
                        )


                a_prep(0)

                # ---- B side (gts): R1, R2, cbc, R3 row0 ----
                for c in range(nu // ct):
                    rows = slice(c * ct * P, (c + 1) * ct * P)
                    dlb = big.tile([P, ct, D], BF16, tag="bigB", bufs=2, name=f"dlb{c}")
                    nc.gpsimd.dma_start(
                        out=dlb, in_=lb[rows, :].rearrange("(p t) f -> p t f", p=P)
                    )
                    dmb = big.tile([P, ct, D], BF16, tag="bigB", bufs=2, name=f"dmb{c}")
                    nc.gpsimd.dma_start(
                        out=dmb, in_=mu_b[rows, :].rearrange("(p t) f -> p t f", p=P)
                    )
                    pf_lb_c = sc.tile([P, ct, P], BF16, tag="ps", bufs=2)
                    nc.sync.dma_start(
                        out=pf_lb_c,
                        in_=dlb.rearrange("p t f -> p (t f)"),
                        transpose=True,
                    )
                    pf_mb_c = sc.tile([P, ct, P], BF16, tag="ps", bufs=2)
                    nc.sync.dma_start(
                        out=pf_mb_c,
                        in_=dmb.rearrange("p t f -> p (t f)"),
                        transpose=True,
                    )
                    # pass 1: exps + muls back-to-back (no cross-engine
                    # round-trips in the Act/DVE queues)
                    cb5s = []
                    for g in range(ct // GT):
                        t0 = c * ct + g * GT
                        ys5 = slice(t0 * P, (t0 + GT) * P)
                        gf = slice(g * GT, (g + 1) * GT)
                        pf_lbf = pf_lb_c[:, gf, :].rearrange("p t f -> p (t f)")
                        pf_mbf = pf_mb_c[:, gf, :].rearrange("p t f -> p (t f)")
                        nc.scalar.activation(R1[:, ys5], pf_lbf, AF.Exp, scale=-1.0)
                        nc.vector.tensor_mul(R2[:, ys5], pf_mbf, R1[:, ys5])
                        m25 = sc.tile([P, GT * P], BF16, tag="sc", bufs=2, name=f"m25_{c}_{g}")
                        nc.vector.tensor_mul(m25, pf_mbf, R2[:, ys5])
                        cb5 = sc.tile([P, GT * P], BF16, tag="cb5", bufs=4, name=f"cb5_{c}_{g}")
                        nc.vector.tensor_add(cb5, m25, pf_lbf)
                        cb5s.append(cb5)
                    # pass 2: c broadcast (ones[128,128] @ cb5) + cbc epilogue
                    for g in range(ct // GT):
                        t0 = c * ct + g * GT
                        ys5 = slice(t0 * P, (t0 + GT) * P)
                        cbc_ps = pso.tile([P, GT * P], F32, tag="po", bufs=2)
                        nc.tensor.matmul(cbc_ps, ones_sq, cb5s[g], start=True, stop=True)
                        nc.scalar.activation(
                            cbc[:, ys5], cbc_ps, AF.Identity, bias=negcm[:, 0:1]
                        )
                    if use3:
                        # R3 row0 = c - CM: copy one broadcast row via DMA
                        nc.sync.dma_start(
                            out=R3[0:1, c * ct * P : (c + 1) * ct * P],
                            in_=cbc[0:1, c * ct * P : (c + 1) * ct * P],
                        )

                # mask -> [P, nt] in the (chunk, partition, tile) x-layout
                nc.sync.dma_start(
                    out=mask_sb.rearrange("p (c t) -> p c t", c=nt // ct),
                    in_=mask.rearrange("(c p t) -> p c t", p=P, t=ct),
                )

                # ---- A side (preds): prep chunk c+1 before GEMM of c ----
                def a_gemm(c):
                    for t in range(c * ct, (c + 1) * ct):
                        xs = slice(t * P, (t + 1) * P)
                        three = use3 and is3(t)
                        for j in range(nyp):
                            pm2 = psm.tile([P, pair * YB], F32, tag="mm")
                            for h in range(pair):
                                n = pair * j + h
                                ysb = slice(n * YB, (n + 1) * YB)
                                dst = pm2[:, h * YB : (h + 1) * YB]
                                nc.tensor.matmul(
                                    dst, L1[:, xs], R1[:, ysb],
                                    start=True, stop=False,
                                )
                                nc.tensor.matmul(
                                    dst, L2[:, xs], R2[:, ysb],
                                    start=False, stop=not three,
                                )
                                if three:
                                    nc.tensor.matmul(
                                        dst, L3[:, xs], R3[:, ysb],
                                        start=False, stop=True,
                                    )
                            ysl2 = slice(pair * j * YB, (pair * j + pair) * YB)
                            slot = slots_all[:, t * nyp + j : t * nyp + j + 1]
                            cp = bfp.tile([P, pair * YB], BF16, tag="cp", bufs=3)
                            nc.scalar.activation(
                                cp, pm2, AF.Identity,
                                bias=(bias3 if three else nbias[:, t : t + 1]),
                            )
                            if three:
                                src = cp
                            else:
                                tfull = bfp.tile([P, pair * YB], BF16, tag="tf", bufs=3)
                                nc.vector.tensor_add(tfull, cp, cbc[:, ysl2])
                                src = tfull
                            junk = bfp.tile([P, pair * YB], BF16, tag="junk", bufs=2)
                            nc.vector.tensor_scalar(
                                out=junk, in0=src, scalar1=BIG, scalar2=None,
                                op0=OP.min, op1=OP.min, accum_out=slot,
                            )
                            nc.vector.tensor_tensor(
                                colmin[:, ysl2], src, colmin[:, ysl2], op=OP.min
                            )

                for c in range(nt // ct):
                    if c + 1 < nt // ct:
                        a_prep(c + 1)
                    a_gemm(c)
                    ts = slice(c * ct, (c + 1) * ct)
                    nc.vector.tensor_reduce(
                        rowmin_all[:, ts],
                        slots_all[:, c * ct * nyp : (c + 1) * ct * nyp].rearrange(
                            "p (t j) -> p t j", j=nyp
                        ),
                        axis=AX.X,
                        op=OP.min,
                    )

            # ---------------- Phase F: final reductions ----------------
            with (
                tc.tile_pool(name="psf", bufs=4, space="PSUM") as psf,
                tc.tile_pool(name="fin", bufs=1) as fin,
            ):
                colmin_f = fin.tile([P, nu], F32)
                FB = 4  # colmin chunks per PSUM tile in the final reduce
                for c4 in range(nu // FB):
                    pc = psf.tile([P, FB, P], BF16, tag="pf", bufs=4)
                    for q in range(FB):
                        cc = c4 * FB + q
                        nc.tensor.transpose(
                            pc[:, q, :], colmin[:, cc * P : (cc + 1) * P], ident_e
                        )
                    nc.vector.tensor_reduce(
                        colmin_f[:, c4 * FB : (c4 + 1) * FB], pc, axis=AX.X,
                        op=OP.min,
                    )
                # loss_1 terms: 0.5*(minT - d) = 0.5*(colmin_f + TREF - D)
                t1 = fin.tile([P, nu], F32)
                nc.vector.tensor_scalar(
                    out=t1, in0=colmin_f, scalar1=TREF - float(D), scalar2=0.5,
                    op0=OP.add, op1=OP.mult,
                )
                l1v = fin.tile([P, 1], F32)
                nc.vector.tensor_reduce(l1v, t1, axis=AX.X, op=OP.add)
                t2 = fin.tile([P, nt], F32)
                nc.vector.tensor_scalar(
                    out=t2, in0=rowmin_all, scalar1=TREF - float(D), scalar2=0.5,
                    op0=OP.add, op1=OP.mult,
                )
                t3 = fin.tile([P, nt], F32)
                nc.vector.tensor_mul(t3, t2, mask_sb)
                l2v = fin.tile([P, 1], F32)
                nc.vector.tensor_reduce(l2v, t3, axis=AX.X, op=OP.add)
                lv = fin.tile([P, 1], F32)
                nc.vector.tensor_add(lv, l1v, l2v)
                p11 = psf.tile([1, 1], F32, tag="p11", bufs=1)
                nc.tensor.matmul(p11, lv, ones_f, start=True, stop=True)
                o_sb = fin.tile([1, 1], F32)
                nc.vector.tensor_copy(o_sb, p11)
                nc.sync.dma_start(out=out_d, in_=o_sb)

        def _set(i):
            return (R1s[i], R2s[i], R3s[i], cbcs[i], L1s[i], L2s[i], L3s[i],
                    colmins[i], rowmins[i], sumlas[i], nbiases[i], slotss[i],
                    masks[i])

        if repeat > 1:
            # unroll by 2 with alternating R buffers: the next half's B-phase
            # feature writes no longer WAR-wait on this half's last GEMM read
            with tc.For_i(0, repeat // 2, 1):
                _phases(*_set(0))
                _phases(*_set(1 % nbuf))
            for _ in range(repeat % 2):
                _phases(*_set(0))
        else:
            _phases(*_set(0))


def _split_waits(nc, limit=1):
    """Hoist excess semaphore waits onto preceding same-engine NoOps.

    The walrus build in this container only supports a small number of sync
    wait commands per hardware instruction (PE self-loading matmuls take just
    one), while Tile freely attaches several.  Equivalent semantics: carriers
    block the engine queue before the instruction executes.
    """
    n = 0
    pe_limit = 1  # S3_LW struct: one wait slot on self-loading matmuls
    for f in nc.m.functions:
        for bb in f.blocks:
            insts = list(bb.instructions)
            out = []
            changed = False
            for inst in insts:
                lim = pe_limit if inst.engine == mybir.EngineType.PE else limit
                si = inst.sync_info
                waits = list(si.on_wait) if (si is not None and si.on_wait) else []
                if len(waits) > lim:
                    for w in waits[:-lim]:
                        n += 1
                        out.append(
                            mybir.InstNoOp(
                                name=f"wsplit-{n}",
                                engine=inst.engine,
                                ins=[],
                                outs=[],
                                sync_info=mybir.SyncInfo(on_wait=[w], on_update=[]),
                            )
                        )
                    si.on_wait = waits[-lim:]
                    changed = True
                out.append(inst)
            if changed:
                bb.instructions = out
    return nc


def build(nx=NX, ny=NY, num_devices=BS, split_waits=True, repeat=1):
    nc = bass.Bass(
        "TRN2", target_bir_lowering=False, debug=False, num_devices=num_devices
    )
    mu_a = nc.dram_tensor("mu_preds", [nx, D], F32, kind="ExternalInput").ap()
    la = nc.dram_tensor("logvar_preds", [nx, D], F32, kind="ExternalInput").ap()
    mu_b = nc.dram_tensor("mu_gts", [ny, D], F32, kind="ExternalInput").ap()
    lb = nc.dram_tensor("logvar_gts", [ny, D], F32, kind="ExternalInput").ap()
    mask = nc.dram_tensor("posterior_mask", [nx], F32, kind="ExternalInput").ap()
    out_d = nc.dram_tensor("loss", [1, 1], F32, kind="ExternalOutput").ap()
    with tile.TileContext(nc) as tc:
        _body(tc, mu_a, la, mu_b, lb, mask, out_d, nx, ny, repeat=repeat)
    if split_waits:
        _split_waits(nc)
    return nc


_NC_CACHE = {}


def _get_nc():
    key = "full"
    if key not in _NC_CACHE:
        _NC_CACHE[key] = build()
    return _NC_CACHE[key]


def kernel_with_stats(trace=False, **inputs):
    nc = _get_nc()
    names = ["mu_preds", "logvar_preds", "mu_gts", "logvar_gts", "posterior_mask"]
    in_maps = [
        {n: np.ascontiguousarray(inputs[n][i], dtype=np.float32) for n in names}
        for i in range(BS)
    ]
    last_err = None
    for attempt in range(3):
        try:
            res = run_bass_kernel_spmd(
                nc, in_maps, core_ids=list(range(BS)), trace=trace
            )
            break
        except Exception as e:  # transient axon/NRT hiccups observed in the wild
            last_err = e
            import time as _time

            _time.sleep(5.0 * (attempt + 1))
    else:
        raise last_err
    out = np.array([res.results[i]["loss"][0, 0] for i in range(BS)], dtype=np.float32)
    return out, res


def kernel(**inputs):
    trace = bool(int(os.environ.get("KERNEL_TRACE", "0")))
    out, _ = kernel_with_stats(trace=trace, **inputs)
    return out
